# revision 23
# baseline (speedup 1.0000x reference)
"""Multi-headed attention (B=4, S=2048, D=1024, H=16) on 8 trn2 NeuronCores.

Sharding: core c handles batch b=c//2, head-half hh=c%2 (heads hh*8..hh*8+7).

Design (v5): the ScalarE exp stream (256 ACTIVATEs x ~1.15us = ~293us/core)
is the hard bottleneck, so the kernel is one continuous j-major (head-pair
outer, query-tile inner) softmax pipeline that keeps ScalarE saturated from
~8us onward. All projection work (K/V/Q/out) is paced background PE work
inside the stream's slack.

Per unit u (= one 256-key chunk-pair cc of a (j,t) tile):
  ScalarE: exp of the two score tiles (head A, head B) of unit u
  PE:      AV (ones-augmented M=65, fused row sums) of unit u-2,
           background projection items, scores matmuls of unit u+1

All SBUF data is split into per-chunk tiles (K/X/Qt per (j,t), V per
128-key chunk, x-staging per t, weights per head-pair slice) so Tile's
dependency tracking is exact — big shared tiles caused tile-granular
false dependencies that serialized the whole ramp on DMA completion.

Inputs are converted to bf16 on the host (no device casts, half the DMA
bytes). log2(e)/8 is folded into wq/bq so exp becomes Exp(ln2 * scores).
Normalization: rowsum row -> reciprocal -> DRAM-roundtrip broadcast DMA
(gpsimd queue) -> in-place X multiply, all off the critical path.

Host: out[b] = core(2b) + core(2b+1) + bo.
"""

from collections import deque

import ml_dtypes
import numpy as np

import concourse.tile as tile
from concourse import bacc, mybir
from concourse.bass_utils import run_bass_kernel_spmd

B, S, D, H = 4, 2048, 1024, 16
HD = D // 2          # feature columns per core (8 heads * 64)
KC = D // 128        # 8 contraction chunks over model dim
FT = HD // 128       # 4 feature tiles (head pairs)
ST = S // 512        # 4 query tiles
RT = S // 128        # 16 key chunks
UTOT = FT * ST * 8   # 128 stream units

f32 = mybir.dt.float32
bf16 = mybir.dt.bfloat16
EXP = mybir.ActivationFunctionType.Exp
LN2 = 0.6931471805599453
LOG2E = 1.4426950408889634

_CACHED_NC = None
_LAST_IN_MAPS = None

MM_NS = 215
ITEM_NS = 4 * MM_NS
UNIT_NS = 2290


def build_nc():
    nc = bacc.Bacc("TRN2", target_bir_lowering=False, debug=False)

    xq_d = nc.dram_tensor("xq", (D, S), bf16, kind="ExternalInput")
    xk_d = nc.dram_tensor("xk", (D, S), bf16, kind="ExternalInput")
    xv_d = nc.dram_tensor("xv", (D, S), bf16, kind="ExternalInput")
    wq_d = nc.dram_tensor("wq", (D, HD), bf16, kind="ExternalInput")
    wk_d = nc.dram_tensor("wk", (D, HD), bf16, kind="ExternalInput")
    wv_d = nc.dram_tensor("wv", (D, HD), bf16, kind="ExternalInput")
    wo_d = nc.dram_tensor("wo", (HD, D), bf16, kind="ExternalInput")
    bqr_d = nc.dram_tensor("bqr", (128, FT), f32, kind="ExternalInput")
    bkr_d = nc.dram_tensor("bkr", (128, FT), f32, kind="ExternalInput")
    bv_d = nc.dram_tensor("bv", (1, HD), bf16, kind="ExternalInput")
    o_d = nc.dram_tensor("o", (S, D), f32, kind="ExternalOutput")

    tsl_of = lambda t: slice(t * 512, (t + 1) * 512)

    with tile.TileContext(nc) as tc:
        with (
            tc.tile_pool(name="cpool", bufs=1) as cpool,
            tc.tile_pool(name="big", bufs=1) as big,
            tc.tile_pool(name="xvs", bufs=14) as xvs,
            tc.tile_pool(name="prp", bufs=6) as prp,
            tc.tile_pool(name="nrm", bufs=1) as nrm,
            tc.tile_pool(name="bcp", bufs=2) as bcp,
            tc.tile_pool(name="ost", bufs=3) as ost,
            tc.tile_pool(name="rsd", bufs=2, space="DRAM") as rsd,
            tc.tile_pool(name="pp", bufs=2, space="PSUM") as pp,
            tc.tile_pool(name="sp", bufs=2, space="PSUM") as sp,
            tc.tile_pool(name="xpp", bufs=1, space="PSUM") as xpp,
        ):
            # ---------------- constants / biases ----------------
            ones_f = cpool.tile([1, 128], f32, name="ones_f")
            nc.gpsimd.memset(ones_f[:], 1.0)
            onesrow = cpool.tile([1, 128], bf16, name="onesrow")
            nc.vector.tensor_copy(onesrow[:], ones_f[:])
            onescol_f = cpool.tile([128, 1], f32, name="onescol_f")
            nc.gpsimd.memset(onescol_f[:], 1.0)
            ones_mm = cpool.tile([128, 1], bf16, name="ones_mm")
            nc.vector.tensor_copy(ones_mm[:], onescol_f[:])
            dscr = cpool.tile([128, 256], bf16, name="dscr")
            nc.gpsimd.memset(dscr[:], 0.0)
            dact_i = cpool.tile([1, 8], f32, name="dact_i")
            nc.gpsimd.memset(dact_i[:], 0.0)
            dact_o = cpool.tile([1, 8], f32, name="dact_o")

            bqr_s = cpool.tile([128, FT], f32, name="bqr_s")
            nc.sync.dma_start(bqr_s[:], bqr_d[:])
            bkr_s = cpool.tile([128, FT], f32, name="bkr_s")
            nc.sync.dma_start(bkr_s[:], bkr_d[:])
            bv_s = cpool.tile([1, HD], bf16, name="bv_s")
            nc.sync.dma_start(bv_s[:], bv_d[:])

            # preload the Exp table while DMAs stream
            nc.scalar.activation(dact_o[:], dact_i[:], EXP, scale=LN2)

            # ---------------- per-chunk SBUF tiles ----------------
            Kt = [[big.tile([128, 512], bf16, name=f"K_{j}_{t}")
                   for t in range(ST)] for j in range(FT)]
            Xt = [[big.tile([128, 512], bf16, name=f"X_{j}_{t}")
                   for t in range(ST)] for j in range(FT)]
            Qt = [[big.tile([128, 512], bf16, name=f"Q_{j}_{t}")
                   for t in range(ST)] for j in range(FT)]
            Vr = [big.tile([128, 8, 65], bf16, name=f"V_{rt}")
                  for rt in range(RT)]
            for rt in range(RT):
                nc.vector.tensor_copy(
                    Vr[rt][:, :, 64:65],
                    onescol_f[:, 0:1].to_broadcast((128, 8, 1)),
                )
            wk_j = [big.tile([128, KC, 128], bf16, name=f"wk_{j}")
                    for j in range(FT)]
            wq_j = [big.tile([128, KC, 128], bf16, name=f"wq_{j}")
                    for j in range(FT)]
            wv_s = big.tile([128, KC, HD], bf16, name="wv_s")
            wo_s = big.tile([128, FT, D], bf16, name="wo_s")
            XKt = [big.tile([128, KC, 512], bf16, name=f"xk_{t}")
                   for t in range(ST)]
            XQt = [big.tile([128, KC, 512], bf16, name=f"xq_{t}")
                   for t in range(ST)]

            # ---------------- input DMAs (ordered by need) ----------------
            xk_r = xk_d[:].rearrange("(k p) s -> p k s", p=128)
            xq_r = xq_d[:].rearrange("(k p) s -> p k s", p=128)

            def wslice(eng, dst, src, ft):
                csl = slice(ft * 128, (ft + 1) * 128)
                eng.dma_start(
                    dst, src[:, csl].rearrange("(k p) n -> p k n", p=128)
                )

            # sync queue: wk(j0), xk tiles, wk rest
            wslice(nc.sync, wk_j[0][:], wk_d, 0)
            for t in range(ST):
                nc.sync.dma_start(XKt[t][:], xk_r[:, :, tsl_of(t)])
            for ft in range(1, FT):
                wslice(nc.sync, wk_j[ft][:], wk_d, ft)

            # gpsimd queue: wq(j0), xq t0, wv, xv groups, xq rest, wq rest, wo
            wslice(nc.gpsimd, wq_j[0][:], wq_d, 0)
            nc.gpsimd.dma_start(XQt[0][:], xq_r[:, :, tsl_of(0)])
            nc.gpsimd.dma_start(
                wv_s[:], wv_d[:].rearrange("(k p) n -> p k n", p=128)
            )

            xv_tiles = {}

            def stage_xv(g):
                gsl = slice(g * 512, (g + 1) * 512)
                tl = []
                for kc in range(KC):
                    xt = xvs.tile([128, 512], bf16, tag="xv", name="xvt")
                    nc.gpsimd.dma_start(
                        xt[:], xv_d[kc * 128 : (kc + 1) * 128, gsl]
                    )
                    tl.append(xt)
                xv_tiles[g] = tl

            stage_xv(0)
            stage_xv(1)
            for t in range(1, ST):
                nc.gpsimd.dma_start(XQt[t][:], xq_r[:, :, tsl_of(t)])
            for ft in range(1, FT):
                wslice(nc.gpsimd, wq_j[ft][:], wq_d, ft)
            nc.gpsimd.dma_start(
                wo_s[:], wo_d[:].rearrange("(f p) d -> p f d", p=128)
            )

            # ---------------- HAM warm-up dummies ----------------
            for i in range(8):
                dps = pp.tile([128, 256], f32, tag="pj", name="dps",
                              padded_shape=[128, 512])
                nc.tensor.matmul(dps[0:1, :], ones_mm[0:128, 0:1], dscr[:],
                                 start=True, stop=True)

            # ---------------- emit helpers ----------------
            def emit_kproj(j, t, half, pend):
                """half 0: kc 0-3 (alloc psum); half 1: kc 4-7 + drain."""
                if half == 0:
                    ps = pp.tile([128, 512], f32, tag="pj", name="pk")
                    pend[(j, t)] = ps
                else:
                    ps = pend.pop((j, t))
                for kc in range(half * 4, half * 4 + 4):
                    nc.tensor.matmul(
                        ps[:], wk_j[j][:, kc, :], XKt[t][:, kc, :],
                        start=(kc == 0), stop=(kc == KC - 1),
                    )
                if half == 1:
                    nc.vector.tensor_scalar_add(
                        Kt[j][t][:], ps[:], bkr_s[:, j : j + 1]
                    )

            def emit_qproj(j, t, half, pend):
                if half == 0:
                    ps = pp.tile([128, 512], f32, tag="pj", name="pq")
                    pend[(j, t)] = ps
                else:
                    ps = pend.pop((j, t))
                for kc in range(half * 4, half * 4 + 4):
                    nc.tensor.matmul(
                        ps[:], wq_j[j][:, kc, :], XQt[t][:, kc, :],
                        start=(kc == 0), stop=(kc == KC - 1),
                    )
                if half == 1:
                    nc.vector.tensor_scalar_add(
                        Qt[j][t][:], ps[:], bqr_s[:, j : j + 1]
                    )

            def emit_vproj(rt):
                g, rr = rt // 4, rt % 4
                ps = pp.tile([128, 512], f32, tag="pj", name="pv")
                rsl = slice(rr * 128, (rr + 1) * 128)
                for kc in range(KC):
                    nc.tensor.matmul(
                        ps[:], xv_tiles[g][kc][:, rsl], wv_s[:, kc, :],
                        start=(kc == 0), stop=False,
                    )
                nc.tensor.matmul(
                    ps[:], onesrow[0:1, :], bv_s[0:1, :],
                    start=False, stop=True,
                )
                nc.vector.tensor_copy(
                    Vr[rt][:, :, 0:64],
                    ps[:].rearrange("p (h e) -> p h e", h=8),
                )

            S_tiles = {}
            P_tiles = {}
            cur = {}

            def emit_scores(u):
                j, t, cc = u // 32, (u // 8) % 4, u % 8
                q = Qt[j][t]
                sA = sp.tile([128, 2, 512], f32, tag="sc", name="sA")
                sB = sp.tile([128, 2, 512], f32, tag="sc", name="sB")
                for hf in range(2):
                    kc = 2 * cc + hf
                    kt, ko = kc // 4, kc % 4
                    ksl = slice(ko * 128, (ko + 1) * 128)
                    nc.tensor.matmul(
                        sA[:, hf, :], Kt[j][kt][0:64, ksl], q[0:64, :],
                        start=True, stop=True, tile_position=(0, 0),
                    )
                    nc.tensor.matmul(
                        sB[:, hf, :], Kt[j][kt][64:128, ksl], q[64:128, :],
                        start=True, stop=True, tile_position=(64, 0),
                    )
                S_tiles[u] = (sA, sB)

            def emit_act(u):
                sA, sB = S_tiles.pop(u)
                pA = prp.tile([128, 2, 512], bf16, tag="pr", name="pA")
                nc.scalar.activation(pA[:], sA[:], EXP, scale=LN2)
                pB = prp.tile([128, 2, 512], bf16, tag="pr", name="pB")
                nc.scalar.activation(pB[:], sB[:], EXP, scale=LN2)
                P_tiles[u] = (pA, pB)

            def emit_avrs(u):
                j, t, cc = u // 32, (u // 8) % 4, u % 8
                pA, pB = P_tiles.pop(u)
                if cc == 0:
                    cur["xpA"] = xpp.tile([65, 512], f32, tag="xpA", name="xpA")
                    cur["xpB"] = xpp.tile([65, 512], f32, tag="xpB", name="xpB")
                xpA, xpB = cur["xpA"], cur["xpB"]
                for hf in range(2):
                    kc = 2 * cc + hf
                    nc.tensor.matmul(
                        xpA[:, :], Vr[kc][:, 2 * j, :], pA[:, hf, :],
                        start=(kc == 0), stop=(kc == RT - 1),
                    )
                    nc.tensor.matmul(
                        xpB[:, :], Vr[kc][:, 2 * j + 1, :], pB[:, hf, :],
                        start=(kc == 0), stop=(kc == RT - 1),
                    )

            def emit_norm(j, t):
                # drain unnormalized X + rowsums, free PSUM immediately;
                # the reciprocal/broadcast/multiply chain completes lazily
                xpA, xpB = cur["xpA"], cur["xpB"]
                xt = Xt[j][t]
                nc.vector.tensor_copy(xt[0:64, :], xpA[0:64, :])
                nc.vector.tensor_copy(xt[64:128, :], xpB[0:64, :])
                rsj = nrm.tile([64, 512], f32, tag="rs", name="rsj")
                nc.vector.tensor_copy(rsj[0:1, :], xpA[64:65, :])
                nc.vector.tensor_copy(rsj[32:33, :], xpB[64:65, :])
                rrh = nrm.tile([64, 512], f32, tag="rr", name="rrh")
                nc.vector.reciprocal_approx_fast(rrh[:], rsj[:])
                rd = rsd.tile([2, 512], f32, tag="rd", name="rd")
                for hh in range(2):
                    nc.gpsimd.dma_start(
                        rd[hh : hh + 1, :], rrh[32 * hh : 32 * hh + 1, :]
                    )
                bcs = bcp.tile([128, 512], f32, tag="bc", name="bcs")
                for hh in range(2):
                    pb = 64 * hh
                    nc.gpsimd.dma_start(
                        bcs[pb : pb + 64, :],
                        rd[hh : hh + 1, :].to_broadcast((64, 512)),
                    )
                    nc.vector.tensor_mul(
                        xt[pb : pb + 64, :],
                        xt[pb : pb + 64, :],
                        bcs[pb : pb + 64, :],
                    )

            def emit_outproj_item(t, r2, n):
                rsl = slice(r2 * 128, (r2 + 1) * 128)
                nsl = slice(n * 512, (n + 1) * 512)
                ps = pp.tile([128, 512], f32, tag="pj", name="po")
                for fc in range(FT):
                    nc.tensor.matmul(
                        ps[:], Xt[fc][t][:, rsl], wo_s[:, fc, nsl],
                        start=(fc == 0), stop=(fc == FT - 1),
                    )
                ot = ost.tile([128, 512], f32, tag="os", name="ot")
                nc.vector.tensor_copy(ot[:], ps[:])
                nc.sync.dma_start(
                    o_d[t * 512 + r2 * 128 : t * 512 + (r2 + 1) * 128, nsl],
                    ot[:],
                )

            # ---------------- background queue ----------------
            bg = deque()
            kpend = {}
            qpend = {}

            def push_kproj(j):
                for t in range(ST):
                    bg.append((ITEM_NS, lambda t=t: emit_kproj(j, t, 0, kpend)))
                    bg.append((ITEM_NS, lambda t=t: emit_kproj(j, t, 1, kpend)))

            def push_outproj(t):
                for r2 in range(4):
                    for n in range(2):
                        bg.append(
                            (ITEM_NS,
                             lambda t=t, r2=r2, n=n: emit_outproj_item(t, r2, n))
                        )

            # ---------------- lead-in ----------------
            emit_kproj(0, 0, 0, kpend)
            emit_kproj(0, 0, 1, kpend)
            emit_qproj(0, 0, 0, qpend)
            emit_qproj(0, 0, 1, qpend)
            emit_scores(0)

            tj0_fixed = {
                0: [lambda: emit_kproj(0, 1, 0, kpend),
                    lambda: emit_kproj(0, 1, 1, kpend),
                    lambda: emit_vproj(0), lambda: emit_vproj(1)],
                1: [lambda: emit_vproj(2), lambda: emit_vproj(3),
                    lambda: stage_xv(2)],
                2: [lambda: emit_kproj(0, 2, 0, kpend),
                    lambda: emit_kproj(0, 2, 1, kpend),
                    lambda: emit_vproj(4), lambda: emit_vproj(5)],
                3: [lambda: emit_vproj(6), lambda: emit_vproj(7),
                    lambda: stage_xv(3)],
                4: [lambda: emit_kproj(0, 3, 0, kpend),
                    lambda: emit_kproj(0, 3, 1, kpend),
                    lambda: emit_vproj(8), lambda: emit_vproj(9)],
                5: [lambda: emit_vproj(10), lambda: emit_vproj(11)],
                6: [lambda: emit_vproj(12), lambda: emit_vproj(13)],
                7: [lambda: emit_vproj(14), lambda: emit_vproj(15)],
            }

            # ---------------- main stream ----------------
            for u in range(UTOT + 2):
                if u < UTOT:
                    emit_act(u)
                j, t, cc = u // 32, (u // 8) % 4, u % 8
                fixed_ns = 0

                if u >= 2:
                    v = u - 2
                    emit_avrs(v)
                    if v % 8 == 7:
                        vj, vt = v // 32, (v // 8) % 4
                        emit_norm(vj, vt)
                        if vj == 3 and vt < 3:
                            push_outproj(vt)

                if u < UTOT:
                    if u == 8:
                        push_kproj(1)
                    elif u == 40:
                        push_kproj(2)
                    elif u == 72:
                        push_kproj(3)

                    # during the DMA-gated ramp, emit scores first so a
                    # DMA-blocked projection can't head-of-line block them
                    if u < 8 and u + 1 < UTOT:
                        emit_scores(u + 1)

                    if u in tj0_fixed:
                        for fn in tj0_fixed[u]:
                            fn()
                        fixed_ns += 2100 * len(tj0_fixed[u])

                    # fixed: Q projection for the next (j,t) — atomic so no
                    # pj-tag psum slot is held across iterations (deadlock)
                    nxt = None
                    if t < ST - 1:
                        nxt = (j, t + 1)
                    elif j < FT - 1:
                        nxt = (j + 1, 0)
                    if nxt is not None and cc == 3:
                        emit_qproj(nxt[0], nxt[1], 0, qpend)
                        emit_qproj(nxt[0], nxt[1], 1, qpend)
                        fixed_ns += 2 * ITEM_NS

                    # background items within remaining PE budget
                    budget = UNIT_NS - 1700 - fixed_ns
                    if j == 3:
                        budget += 1000
                    while bg and budget >= bg[0][0]:
                        cost, fn = bg.popleft()
                        fn()
                        budget -= cost

                    if u >= 8 and u + 1 < UTOT:
                        emit_scores(u + 1)

            while bg:
                _, fn = bg.popleft()
                fn()

            # tail: output projection of the last query tile
            for r2 in range(4):
                for n in range(2):
                    emit_outproj_item(3, r2, n)

    nc.compile()
    return nc


def kernel(**inputs):
    global _CACHED_NC, _LAST_IN_MAPS
    if _CACHED_NC is None:
        _CACHED_NC = build_nc()
    nc = _CACHED_NC

    bfd = ml_dtypes.bfloat16
    query = np.asarray(inputs["query"], dtype=np.float32)
    key = np.asarray(inputs["key"], dtype=np.float32)
    value = np.asarray(inputs["value"], dtype=np.float32)
    fc_w = np.asarray(inputs["fc_w"], dtype=np.float32)
    Wq = np.asarray(inputs["Wq"], dtype=np.float32)
    Wk = np.asarray(inputs["Wk"], dtype=np.float32)
    Wv = np.asarray(inputs["Wv"], dtype=np.float32)
    Wo = np.asarray(inputs["Wo"], dtype=np.float32)
    bq = np.asarray(inputs["bq"], dtype=np.float32)
    bk = np.asarray(inputs["bk"], dtype=np.float32)
    bv = np.asarray(inputs["bv"], dtype=np.float32)
    bo = np.asarray(inputs["bo"], dtype=np.float32)

    qscale = LOG2E / 8.0
    wq_eff = (fc_w * Wq) * qscale

    in_maps = []
    for c in range(8):
        b, hh = c // 2, c % 2
        hs = slice(hh * HD, (hh + 1) * HD)
        in_maps.append({
            "xq": np.ascontiguousarray(query[b].T).astype(bfd),
            "xk": np.ascontiguousarray(key[b].T).astype(bfd),
            "xv": np.ascontiguousarray(value[b].T).astype(bfd),
            "wq": np.ascontiguousarray(wq_eff[:, hs]).astype(bfd),
            "wk": np.ascontiguousarray(Wk[:, hs]).astype(bfd),
            "wv": np.ascontiguousarray(Wv[:, hs]).astype(bfd),
            "wo": np.ascontiguousarray(Wo[hs, :]).astype(bfd),
            "bqr": np.ascontiguousarray(
                (bq[hs] * qscale).reshape(FT, 128).T).astype(np.float32),
            "bkr": np.ascontiguousarray(
                bk[hs].reshape(FT, 128).T).astype(np.float32),
            "bv": bv[None, hs].astype(bfd),
        })

    _LAST_IN_MAPS = in_maps
    res = run_bass_kernel_spmd(nc, in_maps, core_ids=list(range(8)))

    out = np.empty((B, S, D), dtype=np.float32)
    for b in range(B):
        out[b] = res.results[2 * b]["o"] + res.results[2 * b + 1]["o"] + bo
    return out


# revision 30
# speedup vs baseline: 1.0107x; 1.0107x over previous
"""Multi-headed attention (B=4, S=2048, D=1024, H=16) on 8 trn2 NeuronCores.

Sharding: core c handles batch b=c//2, head-half hh=c%2 (heads hh*8..hh*8+7).

Design (v5): the ScalarE exp stream (256 ACTIVATEs x ~1.15us = ~293us/core)
is the hard bottleneck, so the kernel is one continuous j-major (head-pair
outer, query-tile inner) softmax pipeline that keeps ScalarE saturated from
~8us onward. All projection work (K/V/Q/out) is paced background PE work
inside the stream's slack.

Per unit u (= one 256-key chunk-pair cc of a (j,t) tile):
  ScalarE: exp of the two score tiles (head A, head B) of unit u
  PE:      AV (ones-augmented M=65, fused row sums) of unit u-2,
           background projection items, scores matmuls of unit u+1

All SBUF data is split into per-chunk tiles (K/X/Qt per (j,t), V per
128-key chunk, x-staging per t, weights per head-pair slice) so Tile's
dependency tracking is exact — big shared tiles caused tile-granular
false dependencies that serialized the whole ramp on DMA completion.

Inputs are converted to bf16 on the host (no device casts, half the DMA
bytes). log2(e)/8 is folded into wq/bq so exp becomes Exp(ln2 * scores).
Normalization: rowsum row -> reciprocal -> DRAM-roundtrip broadcast DMA
(gpsimd queue) -> in-place X multiply, all off the critical path.

Host: out[b] = core(2b) + core(2b+1) + bo.
"""

from collections import deque

import ml_dtypes
import numpy as np

import concourse.tile as tile
from concourse import bacc, mybir
from concourse.bass_utils import run_bass_kernel_spmd

B, S, D, H = 4, 2048, 1024, 16
HD = D // 2          # feature columns per core (8 heads * 64)
KC = D // 128        # 8 contraction chunks over model dim
FT = HD // 128       # 4 feature tiles (head pairs)
ST = S // 512        # 4 query tiles
RT = S // 128        # 16 key chunks
UTOT = FT * ST * 8   # 128 stream units

f32 = mybir.dt.float32
bf16 = mybir.dt.bfloat16
EXP = mybir.ActivationFunctionType.Exp
LN2 = 0.6931471805599453
LOG2E = 1.4426950408889634

_CACHED_NC = None
_LAST_IN_MAPS = None

MM_NS = 215
ITEM_NS = 4 * MM_NS
UNIT_NS = 2290


def build_nc():
    nc = bacc.Bacc("TRN2", target_bir_lowering=False, debug=False)

    # all inputs host-prearranged partition-major so every DMA moves
    # contiguous multi-KB lines per partition (tiny packets starved the ramp)
    xq_d = nc.dram_tensor("xq", (ST, 128, KC, 512), bf16, kind="ExternalInput")
    xk_d = nc.dram_tensor("xk", (ST, 128, KC, 512), bf16, kind="ExternalInput")
    xv_d = nc.dram_tensor("xv", (ST, 128, KC, 512), bf16, kind="ExternalInput")
    wq_d = nc.dram_tensor("wq", (FT, 128, KC, 128), bf16, kind="ExternalInput")
    wk_d = nc.dram_tensor("wk", (FT, 128, KC, 128), bf16, kind="ExternalInput")
    wv_d = nc.dram_tensor("wv", (128, KC, HD), bf16, kind="ExternalInput")
    wo_d = nc.dram_tensor("wo", (128, FT, D), bf16, kind="ExternalInput")
    bqr_d = nc.dram_tensor("bqr", (128, FT), f32, kind="ExternalInput")
    bkr_d = nc.dram_tensor("bkr", (128, FT), f32, kind="ExternalInput")
    bv_d = nc.dram_tensor("bv", (1, HD), bf16, kind="ExternalInput")
    o_d = nc.dram_tensor("o", (S, D), bf16, kind="ExternalOutput")

    tsl_of = lambda t: slice(t * 512, (t + 1) * 512)

    with tile.TileContext(nc) as tc:
        with (
            tc.tile_pool(name="cpool", bufs=1) as cpool,
            tc.tile_pool(name="big", bufs=1) as big,
            tc.tile_pool(name="xvs", bufs=2) as xvs,
            tc.tile_pool(name="prp", bufs=6) as prp,
            tc.tile_pool(name="nrm", bufs=1) as nrm,
            tc.tile_pool(name="bcp", bufs=2) as bcp,
            tc.tile_pool(name="ost", bufs=3) as ost,
            tc.tile_pool(name="rsd", bufs=2, space="DRAM") as rsd,
            tc.tile_pool(name="pp", bufs=2, space="PSUM") as pp,
            tc.tile_pool(name="sp", bufs=2, space="PSUM") as sp,
            tc.tile_pool(name="xpp", bufs=1, space="PSUM") as xpp,
        ):
            # ---------------- constants / biases ----------------
            ones_f = cpool.tile([1, 128], f32, name="ones_f")
            nc.gpsimd.memset(ones_f[:], 1.0)
            onesrow = cpool.tile([1, 128], bf16, name="onesrow")
            nc.vector.tensor_copy(onesrow[:], ones_f[:])
            onescol_f = cpool.tile([128, 1], f32, name="onescol_f")
            nc.gpsimd.memset(onescol_f[:], 1.0)
            ones_mm = cpool.tile([128, 1], bf16, name="ones_mm")
            nc.vector.tensor_copy(ones_mm[:], onescol_f[:])
            dscr = cpool.tile([128, 256], bf16, name="dscr")
            nc.gpsimd.memset(dscr[:], 0.0)
            dact_i = cpool.tile([1, 8], f32, name="dact_i")
            nc.gpsimd.memset(dact_i[:], 0.0)
            dact_o = cpool.tile([1, 8], f32, name="dact_o")

            bqr_s = cpool.tile([128, FT], f32, name="bqr_s")
            nc.sync.dma_start(bqr_s[:], bqr_d[:])
            bkr_s = cpool.tile([128, FT], f32, name="bkr_s")
            nc.sync.dma_start(bkr_s[:], bkr_d[:])
            bv_s = cpool.tile([1, HD], bf16, name="bv_s")
            nc.sync.dma_start(bv_s[:], bv_d[:])

            # preload the Exp table while DMAs stream
            nc.scalar.activation(dact_o[:], dact_i[:], EXP, scale=LN2)

            # ---------------- per-chunk SBUF tiles ----------------
            Kt = [[big.tile([128, 512], bf16, name=f"K_{j}_{t}")
                   for t in range(ST)] for j in range(FT)]
            Xt = [[big.tile([128, 512], bf16, name=f"X_{j}_{t}")
                   for t in range(ST)] for j in range(FT)]
            Qt = [[big.tile([128, 512], bf16, name=f"Q_{j}_{t}")
                   for t in range(ST)] for j in range(FT)]
            Vr = [big.tile([128, 8, 65], bf16, name=f"V_{rt}")
                  for rt in range(RT)]
            for rt in range(RT):
                nc.vector.tensor_copy(
                    Vr[rt][:, :, 64:65],
                    onescol_f[:, 0:1].to_broadcast((128, 8, 1)),
                )
            wk_j = [big.tile([128, KC, 128], bf16, name=f"wk_{j}")
                    for j in range(FT)]
            wq_j = [big.tile([128, KC, 128], bf16, name=f"wq_{j}")
                    for j in range(FT)]
            wv_s = big.tile([128, KC, HD], bf16, name="wv_s")
            wo_s = big.tile([128, FT, D], bf16, name="wo_s")
            XKt = [big.tile([128, KC, 512], bf16, name=f"xk_{t}")
                   for t in range(ST)]
            XQt = [big.tile([128, KC, 512], bf16, name=f"xq_{t}")
                   for t in range(ST)]

            # ---------------- input DMAs (ordered by need) ----------------
            # sync queue: wk(j0), xk tiles, wk rest
            nc.sync.dma_start(wk_j[0][:], wk_d[0])
            for t in range(ST):
                nc.sync.dma_start(XKt[t][:], xk_d[t])
            for ft in range(1, FT):
                nc.sync.dma_start(wk_j[ft][:], wk_d[ft])

            # gpsimd queue: wq(j0), xq t0, wv, xv groups, xq rest, wq rest, wo
            nc.gpsimd.dma_start(wq_j[0][:], wq_d[0])
            nc.gpsimd.dma_start(XQt[0][:], xq_d[0])
            nc.gpsimd.dma_start(wv_s[:], wv_d[:])

            xv_tiles = {}

            def stage_xv(g):
                xt = xvs.tile([128, KC, 512], bf16, tag="xv", name="xvt")
                nc.gpsimd.dma_start(xt[:], xv_d[g])
                xv_tiles[g] = xt

            stage_xv(0)
            stage_xv(1)
            for t in range(1, ST):
                nc.gpsimd.dma_start(XQt[t][:], xq_d[t])
            for ft in range(1, FT):
                nc.gpsimd.dma_start(wq_j[ft][:], wq_d[ft])
            nc.gpsimd.dma_start(wo_s[:], wo_d[:])

            # ---------------- HAM warm-up dummies ----------------
            for i in range(8):
                dps = pp.tile([128, 256], f32, tag="pj", name="dps",
                              padded_shape=[128, 512])
                nc.tensor.matmul(dps[0:1, :], ones_mm[0:128, 0:1], dscr[:],
                                 start=True, stop=True)

            # ---------------- emit helpers ----------------
            def emit_kproj(j, t, half, pend):
                """half 0: kc 0-3 (alloc psum); half 1: kc 4-7 + drain."""
                if half == 0:
                    ps = pp.tile([128, 512], f32, tag="pj", name="pk")
                    pend[(j, t)] = ps
                else:
                    ps = pend.pop((j, t))
                for kc in range(half * 4, half * 4 + 4):
                    nc.tensor.matmul(
                        ps[:], wk_j[j][:, kc, :], XKt[t][:, kc, :],
                        start=(kc == 0), stop=(kc == KC - 1),
                    )
                if half == 1:
                    nc.vector.tensor_scalar_add(
                        Kt[j][t][:], ps[:], bkr_s[:, j : j + 1]
                    )

            def emit_qproj(j, t, half, pend):
                if half == 0:
                    ps = pp.tile([128, 512], f32, tag="pj", name="pq")
                    pend[(j, t)] = ps
                else:
                    ps = pend.pop((j, t))
                for kc in range(half * 4, half * 4 + 4):
                    nc.tensor.matmul(
                        ps[:], wq_j[j][:, kc, :], XQt[t][:, kc, :],
                        start=(kc == 0), stop=(kc == KC - 1),
                    )
                if half == 1:
                    nc.vector.tensor_scalar_add(
                        Qt[j][t][:], ps[:], bqr_s[:, j : j + 1]
                    )

            def emit_vproj(rt):
                g, rr = rt // 4, rt % 4
                ps = pp.tile([128, 512], f32, tag="pj", name="pv")
                rsl = slice(rr * 128, (rr + 1) * 128)
                for kc in range(KC):
                    nc.tensor.matmul(
                        ps[:], xv_tiles[g][:, kc, rsl], wv_s[:, kc, :],
                        start=(kc == 0), stop=False,
                    )
                nc.tensor.matmul(
                    ps[:], onesrow[0:1, :], bv_s[0:1, :],
                    start=False, stop=True,
                )
                nc.vector.tensor_copy(
                    Vr[rt][:, :, 0:64],
                    ps[:].rearrange("p (h e) -> p h e", h=8),
                )

            S_tiles = {}
            P_tiles = {}
            cur = {}

            def emit_scores(u):
                j, t, cc = u // 32, (u // 8) % 4, u % 8
                q = Qt[j][t]
                sA = sp.tile([128, 2, 512], f32, tag="sc", name="sA")
                sB = sp.tile([128, 2, 512], f32, tag="sc", name="sB")
                for hf in range(2):
                    kc = 2 * cc + hf
                    kt, ko = kc // 4, kc % 4
                    ksl = slice(ko * 128, (ko + 1) * 128)
                    nc.tensor.matmul(
                        sA[:, hf, :], Kt[j][kt][0:64, ksl], q[0:64, :],
                        start=True, stop=True, tile_position=(0, 0),
                    )
                    nc.tensor.matmul(
                        sB[:, hf, :], Kt[j][kt][64:128, ksl], q[64:128, :],
                        start=True, stop=True, tile_position=(64, 0),
                    )
                S_tiles[u] = (sA, sB)

            def emit_act(u):
                sA, sB = S_tiles.pop(u)
                pA = prp.tile([128, 2, 512], bf16, tag="pr", name="pA")
                nc.scalar.activation(pA[:], sA[:], EXP, scale=LN2)
                pB = prp.tile([128, 2, 512], bf16, tag="pr", name="pB")
                nc.scalar.activation(pB[:], sB[:], EXP, scale=LN2)
                P_tiles[u] = (pA, pB)

            def emit_avrs(u):
                j, t, cc = u // 32, (u // 8) % 4, u % 8
                pA, pB = P_tiles.pop(u)
                if cc == 0:
                    cur["xpA"] = xpp.tile([65, 512], f32, tag="xpA", name="xpA")
                    cur["xpB"] = xpp.tile([65, 512], f32, tag="xpB", name="xpB")
                xpA, xpB = cur["xpA"], cur["xpB"]
                for hf in range(2):
                    kc = 2 * cc + hf
                    nc.tensor.matmul(
                        xpA[:, :], Vr[kc][:, 2 * j, :], pA[:, hf, :],
                        start=(kc == 0), stop=(kc == RT - 1),
                    )
                    nc.tensor.matmul(
                        xpB[:, :], Vr[kc][:, 2 * j + 1, :], pB[:, hf, :],
                        start=(kc == 0), stop=(kc == RT - 1),
                    )

            def emit_norm(j, t):
                # drain unnormalized X + rowsums, free PSUM immediately;
                # the reciprocal/broadcast/multiply chain completes lazily
                xpA, xpB = cur["xpA"], cur["xpB"]
                xt = Xt[j][t]
                nc.vector.tensor_copy(xt[0:64, :], xpA[0:64, :])
                nc.vector.tensor_copy(xt[64:128, :], xpB[0:64, :])
                rsj = nrm.tile([64, 512], f32, tag="rs", name="rsj")
                nc.vector.tensor_copy(rsj[0:1, :], xpA[64:65, :])
                nc.vector.tensor_copy(rsj[32:33, :], xpB[64:65, :])
                rrh = nrm.tile([64, 512], f32, tag="rr", name="rrh")
                nc.vector.reciprocal_approx_fast(rrh[:], rsj[:])
                rd = rsd.tile([2, 512], f32, tag="rd", name="rd")
                for hh in range(2):
                    nc.gpsimd.dma_start(
                        rd[hh : hh + 1, :], rrh[32 * hh : 32 * hh + 1, :]
                    )
                bcs = bcp.tile([128, 512], f32, tag="bc", name="bcs")
                for hh in range(2):
                    pb = 64 * hh
                    nc.gpsimd.dma_start(
                        bcs[pb : pb + 64, :],
                        rd[hh : hh + 1, :].to_broadcast((64, 512)),
                    )
                    nc.vector.tensor_mul(
                        xt[pb : pb + 64, :],
                        xt[pb : pb + 64, :],
                        bcs[pb : pb + 64, :],
                    )

            def emit_outproj_item(t, r2, n):
                rsl = slice(r2 * 128, (r2 + 1) * 128)
                nsl = slice(n * 512, (n + 1) * 512)
                ps = pp.tile([128, 512], f32, tag="pj", name="po")
                for fc in range(FT):
                    nc.tensor.matmul(
                        ps[:], Xt[fc][t][:, rsl], wo_s[:, fc, nsl],
                        start=(fc == 0), stop=(fc == FT - 1),
                    )
                ot = ost.tile([128, 512], bf16, tag="os", name="ot")
                nc.vector.tensor_copy(ot[:], ps[:])
                nc.sync.dma_start(
                    o_d[t * 512 + r2 * 128 : t * 512 + (r2 + 1) * 128, nsl],
                    ot[:],
                )

            # ---------------- background queue ----------------
            bg = deque()
            kpend = {}
            qpend = {}

            def push_kproj(j):
                for t in range(ST):
                    bg.append((ITEM_NS, lambda t=t: emit_kproj(j, t, 0, kpend)))
                    bg.append((ITEM_NS, lambda t=t: emit_kproj(j, t, 1, kpend)))

            def push_outproj(t):
                for r2 in range(4):
                    for n in range(2):
                        bg.append(
                            (ITEM_NS,
                             lambda t=t, r2=r2, n=n: emit_outproj_item(t, r2, n))
                        )

            # ---------------- lead-in ----------------
            emit_kproj(0, 0, 0, kpend)
            emit_kproj(0, 0, 1, kpend)
            emit_qproj(0, 0, 0, qpend)
            emit_qproj(0, 0, 1, qpend)
            emit_scores(0)

            tj0_fixed = {
                0: [lambda: emit_kproj(0, 1, 0, kpend),
                    lambda: emit_kproj(0, 1, 1, kpend),
                    lambda: emit_vproj(0), lambda: emit_vproj(1)],
                1: [lambda: emit_vproj(2), lambda: emit_vproj(3),
                    lambda: stage_xv(2)],
                2: [lambda: emit_kproj(0, 2, 0, kpend),
                    lambda: emit_kproj(0, 2, 1, kpend),
                    lambda: emit_vproj(4), lambda: emit_vproj(5)],
                3: [lambda: emit_vproj(6), lambda: emit_vproj(7),
                    lambda: stage_xv(3)],
                4: [lambda: emit_kproj(0, 3, 0, kpend),
                    lambda: emit_kproj(0, 3, 1, kpend),
                    lambda: emit_vproj(8), lambda: emit_vproj(9)],
                5: [lambda: emit_vproj(10), lambda: emit_vproj(11)],
                6: [lambda: emit_vproj(12), lambda: emit_vproj(13)],
                7: [lambda: emit_vproj(14), lambda: emit_vproj(15)],
            }

            # ---------------- main stream ----------------
            for u in range(UTOT + 2):
                if u < UTOT:
                    emit_act(u)
                j, t, cc = u // 32, (u // 8) % 4, u % 8
                fixed_ns = 0

                if u >= 2:
                    v = u - 2
                    emit_avrs(v)
                    if v % 8 == 7:
                        vj, vt = v // 32, (v // 8) % 4
                        emit_norm(vj, vt)
                        if vj == 3 and vt < 3:
                            push_outproj(vt)

                if u < UTOT:
                    if u == 8:
                        push_kproj(1)
                    elif u == 40:
                        push_kproj(2)
                    elif u == 72:
                        push_kproj(3)

                    # during the DMA-gated ramp, emit scores first so a
                    # DMA-blocked projection can't head-of-line block them
                    if u < 8 and u + 1 < UTOT:
                        emit_scores(u + 1)

                    if u in tj0_fixed:
                        for fn in tj0_fixed[u]:
                            fn()
                        fixed_ns += 2100 * len(tj0_fixed[u])

                    # fixed: Q projection for the next (j,t) — atomic so no
                    # pj-tag psum slot is held across iterations (deadlock)
                    nxt = None
                    if t < ST - 1:
                        nxt = (j, t + 1)
                    elif j < FT - 1:
                        nxt = (j + 1, 0)
                    if nxt is not None and cc == 3:
                        emit_qproj(nxt[0], nxt[1], 0, qpend)
                        emit_qproj(nxt[0], nxt[1], 1, qpend)
                        fixed_ns += 2 * ITEM_NS

                    # background items within remaining PE budget
                    budget = UNIT_NS - 1700 - fixed_ns
                    if j == 3:
                        budget += 1900
                    while bg and budget >= bg[0][0]:
                        cost, fn = bg.popleft()
                        fn()
                        budget -= cost

                    if u >= 8 and u + 1 < UTOT:
                        emit_scores(u + 1)

            while bg:
                _, fn = bg.popleft()
                fn()

            # tail: output projection of the last query tile
            for r2 in range(4):
                for n in range(2):
                    emit_outproj_item(3, r2, n)

    nc.compile()
    return nc


def kernel(**inputs):
    global _CACHED_NC, _LAST_IN_MAPS
    if _CACHED_NC is None:
        _CACHED_NC = build_nc()
    nc = _CACHED_NC

    bfd = ml_dtypes.bfloat16
    query = np.asarray(inputs["query"], dtype=np.float32)
    key = np.asarray(inputs["key"], dtype=np.float32)
    value = np.asarray(inputs["value"], dtype=np.float32)
    fc_w = np.asarray(inputs["fc_w"], dtype=np.float32)
    Wq = np.asarray(inputs["Wq"], dtype=np.float32)
    Wk = np.asarray(inputs["Wk"], dtype=np.float32)
    Wv = np.asarray(inputs["Wv"], dtype=np.float32)
    Wo = np.asarray(inputs["Wo"], dtype=np.float32)
    bq = np.asarray(inputs["bq"], dtype=np.float32)
    bk = np.asarray(inputs["bk"], dtype=np.float32)
    bv = np.asarray(inputs["bv"], dtype=np.float32)
    bo = np.asarray(inputs["bo"], dtype=np.float32)

    qscale = LOG2E / 8.0
    wq_eff = (fc_w * Wq) * qscale

    def xarr(xb):
        # [S, D] batch slice -> [ST, 128, KC, 512] partition-major bf16
        return np.ascontiguousarray(
            xb.T.reshape(KC, 128, ST, 512).transpose(2, 1, 0, 3)
        ).astype(bfd)

    def warr(w):
        # [D, HD] -> [FT, 128, KC, 128]
        return np.ascontiguousarray(
            w.reshape(KC, 128, FT, 128).transpose(2, 1, 0, 3)
        ).astype(bfd)

    in_maps = []
    for c in range(8):
        b, hh = c // 2, c % 2
        hs = slice(hh * HD, (hh + 1) * HD)
        in_maps.append({
            "xq": xarr(query[b]),
            "xk": xarr(key[b]),
            "xv": xarr(value[b]),
            "wq": warr(wq_eff[:, hs]),
            "wk": warr(Wk[:, hs]),
            "wv": np.ascontiguousarray(
                Wv[:, hs].reshape(KC, 128, HD).transpose(1, 0, 2)).astype(bfd),
            "wo": np.ascontiguousarray(
                Wo[hs, :].reshape(FT, 128, D).transpose(1, 0, 2)).astype(bfd),
            "bqr": np.ascontiguousarray(
                (bq[hs] * qscale).reshape(FT, 128).T).astype(np.float32),
            "bkr": np.ascontiguousarray(
                bk[hs].reshape(FT, 128).T).astype(np.float32),
            "bv": bv[None, hs].astype(bfd),
        })

    _LAST_IN_MAPS = in_maps
    res = run_bass_kernel_spmd(nc, in_maps, core_ids=list(range(8)))

    out = np.empty((B, S, D), dtype=np.float32)
    for b in range(B):
        out[b] = (res.results[2 * b]["o"].astype(np.float32)
                  + res.results[2 * b + 1]["o"].astype(np.float32) + bo)
    return out
